# revision 13
# baseline (speedup 1.0000x reference)
"""Trainium2 Bass kernel for nn_Attention (Bahdanau-style attention).

Math (per batch row b):
    energy  = tanh(h[b] @ Wh + enc[b] @ We + ba)        # [S, H]
    scores  = energy @ v                                 # [S]
    attn    = softmax(scores)                            # [S]
    context = attn @ enc[b]                              # [2H]

Sharding: data-parallel over batch B=32 across 8 cores (4 rows/core).
Device layout strategy:
  - The big matmul enc @ We contracts over e (=2H): the PE contracts over
    the partition dim, so enc must be laid out [e, s].  We pre-transpose
    enc on the host and upload encT (bf16) per core; a second natural
    copy (bf16) is uploaded for the final context bmm which contracts
    over s.  All accumulation on-device is fp32 (PSUM), so bf16 only
    affects input rounding (~1e-3 rel err).
  - energyT tiles come out [h, s] so the per-row bias (h@Wh + ba) is a
    per-partition scalar -> fused into the tanh activation on ScalarE.
  - scoresT [s_part, 1] columns are produced directly by PE matmuls
    (lhsT = energyT tile, rhs = v chunk), giving softmax the s-on-
    partitions layout that the context bmm needs for its lhsT.
  - softmax skips max-subtraction: |scores| <= ||v||_1 ~ 11.4, exp is
    safe in fp32.
"""

import os

import numpy as np
import ml_dtypes

B, S, H = 32, 2048, 512
E = 2 * H          # 1024
NCORES = 8
BL = B // NCORES   # 4 batch rows per core
P = 128
ST = 512           # s-tile (one PSUM bank of fp32)
NST = S // ST      # 4
EC = E // P        # 8 e-chunks (contraction of the big matmul)
HC = H // P        # 4 h-chunks
SC = S // P        # 16 s-chunks of 128

_BF16 = ml_dtypes.bfloat16

_PROG = None       # cached Bass program
LAST_RESULT = None # BassKernelResults of the last kernel() call (for test.py)


def _emit(ctx, tc, t):
    """Emit the per-core SPMD program. `t` is the dict of dram APs."""
    import concourse.bass as bass  # noqa: F401
    import concourse.mybir as mybir

    nc = tc.nc
    f32 = mybir.dt.float32
    bf16 = mybir.dt.bfloat16
    Tanh = mybir.ActivationFunctionType.Tanh
    Exp = mybir.ActivationFunctionType.Exp
    X = mybir.AxisListType.X

    consts = ctx.enter_context(tc.tile_pool(name="consts", bufs=1))
    io_encn = ctx.enter_context(tc.tile_pool(name="io_encn", bufs=2))
    io_enct = ctx.enter_context(tc.tile_pool(name="io_enct", bufs=3))
    work = ctx.enter_context(tc.tile_pool(name="work", bufs=2))
    ps_energy = ctx.enter_context(tc.tile_pool(name="ps_energy", bufs=2, space="PSUM"))
    ps_row = ctx.enter_context(tc.tile_pool(name="ps_row", bufs=2, space="PSUM"))
    ps_att = ctx.enter_context(tc.tile_pool(name="ps_att", bufs=1, space="PSUM"))
    ps_misc = ctx.enter_context(tc.tile_pool(name="ps_misc", bufs=1, space="PSUM"))
    ps_ctx = ctx.enter_context(tc.tile_pool(name="ps_ctx", bufs=1, space="PSUM"))

    # ---- startup loads.  The first mm1 tile is gated on `we` + the first
    # encT slice, so those two go first on the sync queue; everything else
    # that is not needed immediately is either on another queue (small) or
    # gated behind `we` with a tiny DVE corner-write so its descriptors
    # don't steal SDMA bandwidth from the critical loads.
    we_sb = consts.tile([P, EC, H], bf16)
    nc.sync.dma_start(we_sb[:], t["we"].rearrange("(ec p) h -> p ec h", p=P))
    vt_sb = consts.tile([P, HC, 1], bf16)
    nc.scalar.dma_start(vt_sb[:], t["vt"].rearrange("(hc p) o -> p hc o", p=P))
    wh_sb = consts.tile([P, HC, H], f32)
    nc.gpsimd.dma_start(wh_sb[:], t["wh"].rearrange("(ec p) h -> p ec h", p=P))
    ht_sb = consts.tile([P, HC, BL], f32)
    nc.gpsimd.dma_start(ht_sb[:], t["ht"].rearrange("(ec p) b -> p ec b", p=P))
    ba_sb = consts.tile([P, HC * BL], f32)
    nc.gpsimd.dma_start(ba_sb[:], t["barep"])
    ones_row = consts.tile([1, P], f32)
    nc.gpsimd.dma_start(ones_row[:], t["ones_row"])
    ones_col = consts.tile([P, 1], f32)
    nc.gpsimd.dma_start(ones_col[:], t["ones_col"])
    id128 = consts.tile([P, P], f32)
    nc.gpsimd.dma_start(id128[:], t["id128"])
    onesb = consts.tile([1, 1], bf16)
    nc.vector.memset(onesb[:], 1.0)
    onescol_b = consts.tile([P, 1], bf16)
    nc.vector.memset(onescol_b[:], 1.0)

    # PE warm-up: a few dependency-free matmuls run during the first DMA
    # wait and trip the HAM clock gate to 2.4 GHz before real work starts.
    scratch = consts.tile([P, ST], bf16)
    nc.vector.memset(scratch[:], 0.25)
    warm_ps = ps_misc.tile([P, 512], f32, tag="misc", name="warm_ps")
    for _ in range(8):
        nc.tensor.matmul(warm_ps[:], scratch[:, 0:P], scratch[:], start=True, stop=True)

    encn_tiles = [None] * BL
    eng_tiles = {}
    exprow_tiles = [None] * BL
    exprow_b16_tiles = [None] * BL
    attnT_ps_tiles = [None] * BL
    attnb_tiles = [None] * BL
    cps_tiles = [None] * BL
    bias_sb = consts.tile([P, HC * BL], f32)

    encT_r = t["enct"].rearrange("b (ec p) s -> b p ec s", p=P)
    encN_r = t["encn"].rearrange("b (sc p) e -> b p sc e", p=P)
    attn_r = t["attn_out"].rearrange("b (q p) -> b q p", p=P)

    def emit_hproj():
        # h_proj[b, h] via a single M=4 matmul chain, then transpose to
        # [h, b] columns for the per-partition tanh bias.
        hp_ps = ps_misc.tile([P, 512], f32, tag="misc", name="hp_ps")
        for ec in range(HC):
            nc.tensor.matmul(
                hp_ps[0:BL, 0:H],
                ht_sb[:, ec, :],
                wh_sb[:, ec, :],
                start=(ec == 0),
                stop=(ec == HC - 1),
            )
        hp_sb = consts.tile([BL, H], f32)
        nc.vector.tensor_copy(hp_sb[:], hp_ps[0:BL, 0:H])
        hpT_ps = ps_misc.tile([P, 512], f32, tag="misc", name="hpT_ps")
        for hc in range(HC):
            nc.tensor.transpose(
                hpT_ps[:, hc * BL:(hc + 1) * BL],
                hp_sb[:, hc * P:(hc + 1) * P],
                id128[0:BL, 0:BL],
            )
        nc.vector.tensor_add(bias_sb[:], hpT_ps[:, 0:HC * BL], ba_sb[:])

    def emit_mm1(b, st):
        et = io_enct.tile([P, EC, ST], bf16, tag="et", name=f"et{b}_{st}")
        if b == 0 and st in (1, 2):
            nc.vector.tensor_copy(et[0:1, 0, 0:1], we_sb[0:1, 0, 0:1])
        nc.sync.dma_start(et[:], encT_r[b, :, :, st * ST:(st + 1) * ST])
        eng = work.tile([P, HC, ST], bf16, tag="eng", bufs=3, name=f"eng{b}_{st}")
        eng_tiles[(b, st)] = eng
        for hc in range(HC):
            pe = ps_energy.tile([P, ST], f32, tag="pe", name=f"pe{b}_{st}_{hc}")
            for ec in range(EC):
                nc.tensor.matmul(
                    pe[:],
                    we_sb[:, ec, hc * P:(hc + 1) * P],
                    et[:, ec, :],
                    start=(ec == 0),
                    stop=(ec == EC - 1),
                )
            if b == 0 and st == 0 and hc == 0:
                emit_hproj()
            nc.scalar.activation(
                eng[:, hc, :], pe[:], Tanh,
                bias=bias_sb[:, hc * BL + b:hc * BL + b + 1],
            )

    def emit_scores(b, st):
        # scores row [1, 512] = v.T @ energyT (v stationary: trivial LDW),
        # exp on ScalarE, bf16 cast, then per-chunk transposes to columns.
        row = ps_row.tile([1, ST], f32, tag="row", name=f"row{b}_{st}")
        for hc in range(HC):
            nc.tensor.matmul(
                row[:],
                vt_sb[:, hc, :],
                eng_tiles[(b, st)][:, hc, :],
                start=(hc == 0),
                stop=(hc == HC - 1),
            )
        cols = slice(st * ST, (st + 1) * ST)
        nc.scalar.activation(exprow_tiles[b][:, cols], row[:], Exp)
        nc.vector.tensor_copy(exprow_b16_tiles[b][:, cols], exprow_tiles[b][:, cols])
        for j in range(NST):
            sc = st * NST + j
            nc.tensor.transpose(
                attnT_ps_tiles[b][:, 2 * sc:2 * sc + 1],
                exprow_b16_tiles[b][0:1, sc * P:(sc + 1) * P],
                onesb[:],
            )
        nc.vector.tensor_copy(
            attnb_tiles[b][:, st * NST:(st + 1) * NST],
            attnT_ps_tiles[b][:, 2 * st * NST:2 * (st + 1) * NST:2],
        )

    def emit_bmm2_chunk(b, st):
        # unnormalized context accumulation; scaled by 1/sum at the end.
        for j in range(NST):
            sc = st * NST + j
            for eh in range(2):
                nc.tensor.matmul(
                    cps_tiles[b][:, eh * ST:(eh + 1) * ST],
                    attnb_tiles[b][:, sc:sc + 1],
                    encn_tiles[b][:, sc, eh * ST:(eh + 1) * ST],
                    start=(sc == 0),
                    stop=(sc == SC - 1),
                )

    def emit_softmax_finish(b):
        misc = ps_misc.tile([P, 512], f32, tag="misc", name=f"misc{b}")
        nc.tensor.matmul(
            misc[0:1, 0:SC], onescol_b[:], attnb_tiles[b][:], start=True, stop=True
        )
        ssum = work.tile([1, 1], f32, tag="ssum", name=f"ssum{b}")
        nc.vector.reduce_sum(ssum[:], misc[0:1, 0:SC], axis=X)
        sinv = work.tile([1, 1], f32, tag="sinv", name=f"sinv{b}")
        nc.vector.reciprocal(sinv[:], ssum[:])
        # context: scale the accumulated bmm2 by 1/sum on the way out
        csb = work.tile([1, E], f32, tag="csb", name=f"csb{b}")
        nc.vector.tensor_scalar_mul(csb[:], cps_tiles[b][:], sinv[:])
        nc.sync.dma_start(t["ctx_out"][b:b + 1, :], csb[:])
        # attention-weights output row
        nc.tensor.matmul(misc[:, 16:17], ones_row[:], sinv[:], start=True, stop=True)
        inv_col = work.tile([P, 1], f32, tag="inv_col", name=f"inv{b}")
        nc.vector.tensor_copy(inv_col[:], misc[:, 16:17])
        attnT_f = work.tile([P, SC], f32, tag="attnT_f", name=f"attnTf{b}")
        nc.vector.tensor_scalar_mul(
            attnT_f[:], attnT_ps_tiles[b][:, 0:2 * SC:2], inv_col[:]
        )
        nc.tensor.transpose(misc[0:SC, 17:17 + P], attnT_f[:], id128[:])
        att_sb = work.tile([SC, P], f32, tag="att_sb", name=f"att{b}")
        nc.vector.tensor_copy(att_sb[:], misc[0:SC, 17:17 + P])
        nc.sync.dma_start(attn_r[b], att_sb[:])

    for b in range(BL):
        encn_t = io_encn.tile([P, SC, E], bf16, tag="encn", name=f"encn{b}")
        if b == 0:
            for q in range(NST):
                nc.vector.tensor_copy(encn_t[0:1, q * NST, 0:1], we_sb[0:1, 0, 0:1])
        for q in range(NST):
            nc.scalar.dma_start(
                encn_t[:, q * NST:(q + 1) * NST, :],
                encN_r[b, :, q * NST:(q + 1) * NST, :],
            )
        encn_tiles[b] = encn_t
        exprow_tiles[b] = work.tile([1, S], f32, tag="exprow", name=f"exprow{b}")
        exprow_b16_tiles[b] = work.tile([1, S], bf16, tag="exprowb", name=f"exprowb{b}")
        attnT_ps_tiles[b] = ps_att.tile([P, 2 * SC], bf16, tag="attps", name=f"attps{b}")
        attnb_tiles[b] = work.tile([P, SC], bf16, tag="attnb", name=f"attnb{b}")
        cps_tiles[b] = ps_ctx.tile([1, E], f32, tag="cps", name=f"cps{b}")
        for st in range(NST):
            emit_mm1(b, st)
            if st == 1 and b > 0:
                emit_softmax_finish(b - 1)
            if st > 0:
                emit_scores(b, st - 1)
                emit_bmm2_chunk(b, st - 1)
        emit_scores(b, NST - 1)
        emit_bmm2_chunk(b, NST - 1)
    emit_softmax_finish(BL - 1)


def _build_program():
    from contextlib import ExitStack

    import concourse.mybir as mybir
    import concourse.tile as tile
    from concourse import bacc

    f32 = mybir.dt.float32
    bf16 = mybir.dt.bfloat16

    nc = bacc.Bacc("TRN2", debug=False, num_devices=NCORES)
    t = {
        "enct": nc.dram_tensor("enct", [BL, E, S], bf16, kind="ExternalInput").ap(),
        "encn": nc.dram_tensor("encn", [BL, S, E], bf16, kind="ExternalInput").ap(),
        "we": nc.dram_tensor("we", [E, H], bf16, kind="ExternalInput").ap(),
        "wh": nc.dram_tensor("wh", [H, H], f32, kind="ExternalInput").ap(),
        "ht": nc.dram_tensor("ht", [H, BL], f32, kind="ExternalInput").ap(),
        "barep": nc.dram_tensor("barep", [P, HC * BL], f32, kind="ExternalInput").ap(),
        "vt": nc.dram_tensor("vt", [H, 1], bf16, kind="ExternalInput").ap(),
        "ones_row": nc.dram_tensor("ones_row", [1, P], f32, kind="ExternalInput").ap(),
        "ones_col": nc.dram_tensor("ones_col", [P, 1], f32, kind="ExternalInput").ap(),
        "id128": nc.dram_tensor("id128", [P, P], f32, kind="ExternalInput").ap(),
        "ctx_out": nc.dram_tensor("ctx_out", [BL, E], f32, kind="ExternalOutput").ap(),
        "attn_out": nc.dram_tensor("attn_out", [BL, S], f32, kind="ExternalOutput").ap(),
    }
    with tile.TileContext(nc) as tc, ExitStack() as ctx:
        _emit(ctx, tc, t)
    nc.compile()
    return nc


def get_program():
    global _PROG
    if _PROG is None:
        _PROG = _build_program()
    return _PROG


def make_in_maps(hidden, encoder_outputs, Wa, ba, v):
    hidden = np.asarray(hidden, dtype=np.float32)
    enc = np.asarray(encoder_outputs, dtype=np.float32)
    Wa = np.asarray(Wa, dtype=np.float32)
    ba = np.asarray(ba, dtype=np.float32)
    v = np.asarray(v, dtype=np.float32)

    h = hidden[0]                       # [B, H]
    we = np.ascontiguousarray(Wa[H:]).astype(_BF16)         # [E, H]
    wh = np.ascontiguousarray(Wa[:H])                       # [H, H] f32
    ba_rep = np.ascontiguousarray(
        np.repeat(ba.reshape(HC, P).T, BL, axis=1)          # [128, HC*BL]
    ).astype(np.float32)
    vt = np.ascontiguousarray(v.reshape(H, 1)).astype(_BF16)
    ones_row = np.ones((1, P), np.float32)
    ones_col = np.ones((P, 1), np.float32)
    id128 = np.eye(P, dtype=np.float32)

    in_maps = []
    for c in range(NCORES):
        rows = slice(c * BL, (c + 1) * BL)
        encc = enc[rows]
        in_maps.append({
            "enct": np.ascontiguousarray(encc.transpose(0, 2, 1)).astype(_BF16),
            "encn": encc.astype(_BF16),
            "we": we,
            "wh": wh,
            "ht": np.ascontiguousarray(h[rows].T).astype(np.float32),
            "barep": ba_rep,
            "vt": vt,
            "ones_row": ones_row,
            "ones_col": ones_col,
            "id128": id128,
        })
    return in_maps


def kernel(hidden, encoder_outputs, Wa, ba, v):
    global LAST_RESULT
    from concourse import bass_utils

    nc = get_program()
    in_maps = make_in_maps(hidden, encoder_outputs, Wa, ba, v)
    trace = bool(int(os.environ.get("BASS_TRACE", "0") or "0"))
    res = bass_utils.run_bass_kernel_spmd(
        nc, in_maps, core_ids=list(range(NCORES)), trace=trace
    )
    LAST_RESULT = res
    context = np.concatenate([r["ctx_out"] for r in res.results], axis=0)
    attn = np.concatenate([r["attn_out"] for r in res.results], axis=0)
    return context, attn


# revision 14
# speedup vs baseline: 1.0414x; 1.0414x over previous
"""Trainium2 Bass kernel for nn_Attention (Bahdanau-style attention).

Math (per batch row b):
    energy  = tanh(h[b] @ Wh + enc[b] @ We + ba)        # [S, H]
    scores  = energy @ v                                 # [S]
    attn    = softmax(scores)                            # [S]
    context = attn @ enc[b]                              # [2H]

Sharding: data-parallel over batch B=32 across 8 cores (4 rows/core).
Device layout strategy:
  - The big matmul enc @ We contracts over e (=2H): the PE contracts over
    the partition dim, so enc must be laid out [e, s].  We pre-transpose
    enc on the host and upload encT (bf16) per core; a second natural
    copy (bf16) is uploaded for the final context bmm which contracts
    over s.  All accumulation on-device is fp32 (PSUM), so bf16 only
    affects input rounding (~1e-3 rel err).
  - energyT tiles come out [h, s] so the per-row bias (h@Wh + ba) is a
    per-partition scalar -> fused into the tanh activation on ScalarE.
  - scoresT [s_part, 1] columns are produced directly by PE matmuls
    (lhsT = energyT tile, rhs = v chunk), giving softmax the s-on-
    partitions layout that the context bmm needs for its lhsT.
  - softmax skips max-subtraction: |scores| <= ||v||_1 ~ 11.4, exp is
    safe in fp32.
"""

import os

import numpy as np
import ml_dtypes

B, S, H = 32, 2048, 512
E = 2 * H          # 1024
NCORES = 8
BL = B // NCORES   # 4 batch rows per core
P = 128
ST = 512           # s-tile (one PSUM bank of fp32)
NST = S // ST      # 4
EC = E // P        # 8 e-chunks (contraction of the big matmul)
HC = H // P        # 4 h-chunks
SC = S // P        # 16 s-chunks of 128

_BF16 = ml_dtypes.bfloat16

_PROG = None       # cached Bass program
LAST_RESULT = None # BassKernelResults of the last kernel() call (for test.py)


def _emit(ctx, tc, t):
    """Emit the per-core SPMD program. `t` is the dict of dram APs."""
    import concourse.bass as bass  # noqa: F401
    import concourse.mybir as mybir

    nc = tc.nc
    f32 = mybir.dt.float32
    bf16 = mybir.dt.bfloat16
    Tanh = mybir.ActivationFunctionType.Tanh
    Exp = mybir.ActivationFunctionType.Exp
    X = mybir.AxisListType.X

    consts = ctx.enter_context(tc.tile_pool(name="consts", bufs=1))
    io_encn = ctx.enter_context(tc.tile_pool(name="io_encn", bufs=2))
    io_enct = ctx.enter_context(tc.tile_pool(name="io_enct", bufs=3))
    work = ctx.enter_context(tc.tile_pool(name="work", bufs=2))
    ps_energy = ctx.enter_context(tc.tile_pool(name="ps_energy", bufs=2, space="PSUM"))
    ps_row = ctx.enter_context(tc.tile_pool(name="ps_row", bufs=2, space="PSUM"))
    ps_att = ctx.enter_context(tc.tile_pool(name="ps_att", bufs=1, space="PSUM"))
    ps_misc = ctx.enter_context(tc.tile_pool(name="ps_misc", bufs=1, space="PSUM"))
    ps_ctx = ctx.enter_context(tc.tile_pool(name="ps_ctx", bufs=1, space="PSUM"))

    # ---- startup loads.  The first mm1 tile is gated on `we` + the first
    # encT slice, so those two go first on the sync queue; everything else
    # that is not needed immediately is either on another queue (small) or
    # gated behind `we` with a tiny DVE corner-write so its descriptors
    # don't steal SDMA bandwidth from the critical loads.
    we_sb = consts.tile([P, EC, H], bf16)
    nc.sync.dma_start(we_sb[:], t["we"].rearrange("(ec p) h -> p ec h", p=P))
    vt_sb = consts.tile([P, HC, 1], bf16)
    nc.scalar.dma_start(vt_sb[:], t["vt"].rearrange("(hc p) o -> p hc o", p=P))
    wh_sb = consts.tile([P, HC, H], f32)
    nc.scalar.dma_start(wh_sb[:], t["wh"].rearrange("(ec p) h -> p ec h", p=P))
    ht_sb = consts.tile([P, HC, BL], f32)
    nc.scalar.dma_start(ht_sb[:], t["ht"].rearrange("(ec p) b -> p ec b", p=P))
    ba_sb = consts.tile([P, HC * BL], f32)
    nc.scalar.dma_start(ba_sb[:], t["barep"])
    ones_row = consts.tile([1, P], f32)
    nc.scalar.dma_start(ones_row[:], t["ones_row"])
    ones_col = consts.tile([P, 1], f32)
    nc.scalar.dma_start(ones_col[:], t["ones_col"])
    id128 = consts.tile([P, P], f32)
    nc.scalar.dma_start(id128[:], t["id128"])
    onesb = consts.tile([1, 1], bf16)
    nc.vector.memset(onesb[:], 1.0)
    onescol_b = consts.tile([P, 1], bf16)
    nc.vector.memset(onescol_b[:], 1.0)

    # PE warm-up: a few dependency-free matmuls run during the first DMA
    # wait and trip the HAM clock gate to 2.4 GHz before real work starts.
    scratch = consts.tile([P, ST], bf16)
    nc.vector.memset(scratch[:], 0.25)
    warm_ps = ps_misc.tile([P, 512], f32, tag="misc", name="warm_ps")
    for _ in range(8):
        nc.tensor.matmul(warm_ps[:], scratch[:, 0:P], scratch[:], start=True, stop=True)

    encn_tiles = [None] * BL
    eng_tiles = {}
    exprow_tiles = [None] * BL
    exprow_b16_tiles = [None] * BL
    attnT_ps_tiles = [None] * BL
    attnb_tiles = [None] * BL
    cps_tiles = [None] * BL
    bias_sb = consts.tile([P, HC * BL], f32)

    encT_r = t["enct"].rearrange("b (ec p) s -> b p ec s", p=P)
    encN_r = t["encn"].rearrange("b (sc p) e -> b p sc e", p=P)
    attn_r = t["attn_out"].rearrange("b (q p) -> b q p", p=P)

    def emit_hproj():
        # h_proj[b, h] via a single M=4 matmul chain, then transpose to
        # [h, b] columns for the per-partition tanh bias.
        hp_ps = ps_misc.tile([P, 512], f32, tag="misc", name="hp_ps")
        for ec in range(HC):
            nc.tensor.matmul(
                hp_ps[0:BL, 0:H],
                ht_sb[:, ec, :],
                wh_sb[:, ec, :],
                start=(ec == 0),
                stop=(ec == HC - 1),
            )
        hp_sb = consts.tile([BL, H], f32)
        nc.vector.tensor_copy(hp_sb[:], hp_ps[0:BL, 0:H])
        hpT_ps = ps_misc.tile([P, 512], f32, tag="misc", name="hpT_ps")
        for hc in range(HC):
            nc.tensor.transpose(
                hpT_ps[:, hc * BL:(hc + 1) * BL],
                hp_sb[:, hc * P:(hc + 1) * P],
                id128[0:BL, 0:BL],
            )
        nc.vector.tensor_add(bias_sb[:], hpT_ps[:, 0:HC * BL], ba_sb[:])

    def emit_mm1(b, st):
        et = io_enct.tile([P, EC, ST], bf16, tag="et", name=f"et{b}_{st}")
        if b == 0 and st in (1, 2):
            nc.vector.tensor_copy(et[0:1, 0, 0:1], we_sb[0:1, 0, 0:1])
        nc.sync.dma_start(et[:], encT_r[b, :, :, st * ST:(st + 1) * ST])
        eng = work.tile([P, HC, ST], bf16, tag="eng", bufs=3, name=f"eng{b}_{st}")
        eng_tiles[(b, st)] = eng
        for hc in range(HC):
            pe = ps_energy.tile([P, ST], f32, tag="pe", name=f"pe{b}_{st}_{hc}")
            for ec in range(EC):
                nc.tensor.matmul(
                    pe[:],
                    we_sb[:, ec, hc * P:(hc + 1) * P],
                    et[:, ec, :],
                    start=(ec == 0),
                    stop=(ec == EC - 1),
                )
            if b == 0 and st == 0 and hc == 0:
                emit_hproj()
            nc.scalar.activation(
                eng[:, hc, :], pe[:], Tanh,
                bias=bias_sb[:, hc * BL + b:hc * BL + b + 1],
            )

    def emit_scores(b, st):
        # scores row [1, 512] = v.T @ energyT (v stationary: trivial LDW),
        # exp on ScalarE, bf16 cast, then per-chunk transposes to columns.
        row = ps_row.tile([1, ST], f32, tag="row", name=f"row{b}_{st}")
        for hc in range(HC):
            nc.tensor.matmul(
                row[:],
                vt_sb[:, hc, :],
                eng_tiles[(b, st)][:, hc, :],
                start=(hc == 0),
                stop=(hc == HC - 1),
            )
        cols = slice(st * ST, (st + 1) * ST)
        nc.scalar.activation(exprow_tiles[b][:, cols], row[:], Exp)
        nc.vector.tensor_copy(exprow_b16_tiles[b][:, cols], exprow_tiles[b][:, cols])
        for j in range(NST):
            sc = st * NST + j
            nc.tensor.transpose(
                attnT_ps_tiles[b][:, 2 * sc:2 * sc + 1],
                exprow_b16_tiles[b][0:1, sc * P:(sc + 1) * P],
                onesb[:],
            )
        nc.vector.tensor_copy(
            attnb_tiles[b][:, st * NST:(st + 1) * NST],
            attnT_ps_tiles[b][:, 2 * st * NST:2 * (st + 1) * NST:2],
        )

    def emit_bmm2_chunk(b, st):
        # unnormalized context accumulation; scaled by 1/sum at the end.
        for j in range(NST):
            sc = st * NST + j
            for eh in range(2):
                nc.tensor.matmul(
                    cps_tiles[b][:, eh * ST:(eh + 1) * ST],
                    attnb_tiles[b][:, sc:sc + 1],
                    encn_tiles[b][:, sc, eh * ST:(eh + 1) * ST],
                    start=(sc == 0),
                    stop=(sc == SC - 1),
                )

    def emit_softmax_finish(b):
        misc = ps_misc.tile([P, 512], f32, tag="misc", name=f"misc{b}")
        nc.tensor.matmul(
            misc[0:1, 0:SC], onescol_b[:], attnb_tiles[b][:], start=True, stop=True
        )
        ssum = work.tile([1, 1], f32, tag="ssum", name=f"ssum{b}")
        nc.vector.reduce_sum(ssum[:], misc[0:1, 0:SC], axis=X)
        sinv = work.tile([1, 1], f32, tag="sinv", name=f"sinv{b}")
        nc.vector.reciprocal(sinv[:], ssum[:])
        # context: scale the accumulated bmm2 by 1/sum on the way out
        csb = work.tile([1, E], f32, tag="csb", name=f"csb{b}")
        nc.vector.tensor_scalar_mul(csb[:], cps_tiles[b][:], sinv[:])
        nc.sync.dma_start(t["ctx_out"][b:b + 1, :], csb[:])
        # attention-weights output row
        nc.tensor.matmul(misc[:, 16:17], ones_row[:], sinv[:], start=True, stop=True)
        inv_col = work.tile([P, 1], f32, tag="inv_col", name=f"inv{b}")
        nc.vector.tensor_copy(inv_col[:], misc[:, 16:17])
        attnT_f = work.tile([P, SC], f32, tag="attnT_f", name=f"attnTf{b}")
        nc.vector.tensor_scalar_mul(
            attnT_f[:], attnT_ps_tiles[b][:, 0:2 * SC:2], inv_col[:]
        )
        nc.tensor.transpose(misc[0:SC, 17:17 + P], attnT_f[:], id128[:])
        att_sb = work.tile([SC, P], f32, tag="att_sb", name=f"att{b}")
        nc.vector.tensor_copy(att_sb[:], misc[0:SC, 17:17 + P])
        nc.sync.dma_start(attn_r[b], att_sb[:])

    for b in range(BL):
        encn_t = io_encn.tile([P, SC, E], bf16, tag="encn", name=f"encn{b}")
        if b == 0:
            for q in range(NST):
                nc.vector.tensor_copy(encn_t[0:1, q * NST, 0:1], we_sb[0:1, 0, 0:1])
        elif b == 1:
            for q in range(NST):
                nc.vector.tensor_copy(
                    encn_t[0:1, q * NST, 0:1], eng_tiles[(0, 0)][0:1, 0, 0:1]
                )
        for q in range(NST):
            nc.scalar.dma_start(
                encn_t[:, q * NST:(q + 1) * NST, :],
                encN_r[b, :, q * NST:(q + 1) * NST, :],
            )
        encn_tiles[b] = encn_t
        exprow_tiles[b] = work.tile([1, S], f32, tag="exprow", name=f"exprow{b}")
        exprow_b16_tiles[b] = work.tile([1, S], bf16, tag="exprowb", name=f"exprowb{b}")
        attnT_ps_tiles[b] = ps_att.tile([P, 2 * SC], bf16, tag="attps", name=f"attps{b}")
        attnb_tiles[b] = work.tile([P, SC], bf16, tag="attnb", name=f"attnb{b}")
        cps_tiles[b] = ps_ctx.tile([1, E], f32, tag="cps", name=f"cps{b}")
        for st in range(NST):
            emit_mm1(b, st)
            if st == 1 and b > 0:
                emit_softmax_finish(b - 1)
            if st > 0:
                emit_scores(b, st - 1)
                emit_bmm2_chunk(b, st - 1)
        emit_scores(b, NST - 1)
        emit_bmm2_chunk(b, NST - 1)
    emit_softmax_finish(BL - 1)


def _build_program():
    from contextlib import ExitStack

    import concourse.mybir as mybir
    import concourse.tile as tile
    from concourse import bacc

    f32 = mybir.dt.float32
    bf16 = mybir.dt.bfloat16

    nc = bacc.Bacc("TRN2", debug=False, num_devices=NCORES)
    t = {
        "enct": nc.dram_tensor("enct", [BL, E, S], bf16, kind="ExternalInput").ap(),
        "encn": nc.dram_tensor("encn", [BL, S, E], bf16, kind="ExternalInput").ap(),
        "we": nc.dram_tensor("we", [E, H], bf16, kind="ExternalInput").ap(),
        "wh": nc.dram_tensor("wh", [H, H], f32, kind="ExternalInput").ap(),
        "ht": nc.dram_tensor("ht", [H, BL], f32, kind="ExternalInput").ap(),
        "barep": nc.dram_tensor("barep", [P, HC * BL], f32, kind="ExternalInput").ap(),
        "vt": nc.dram_tensor("vt", [H, 1], bf16, kind="ExternalInput").ap(),
        "ones_row": nc.dram_tensor("ones_row", [1, P], f32, kind="ExternalInput").ap(),
        "ones_col": nc.dram_tensor("ones_col", [P, 1], f32, kind="ExternalInput").ap(),
        "id128": nc.dram_tensor("id128", [P, P], f32, kind="ExternalInput").ap(),
        "ctx_out": nc.dram_tensor("ctx_out", [BL, E], f32, kind="ExternalOutput").ap(),
        "attn_out": nc.dram_tensor("attn_out", [BL, S], f32, kind="ExternalOutput").ap(),
    }
    with tile.TileContext(nc) as tc, ExitStack() as ctx:
        _emit(ctx, tc, t)
    nc.compile()
    return nc


def get_program():
    global _PROG
    if _PROG is None:
        _PROG = _build_program()
    return _PROG


def make_in_maps(hidden, encoder_outputs, Wa, ba, v):
    hidden = np.asarray(hidden, dtype=np.float32)
    enc = np.asarray(encoder_outputs, dtype=np.float32)
    Wa = np.asarray(Wa, dtype=np.float32)
    ba = np.asarray(ba, dtype=np.float32)
    v = np.asarray(v, dtype=np.float32)

    h = hidden[0]                       # [B, H]
    we = np.ascontiguousarray(Wa[H:]).astype(_BF16)         # [E, H]
    wh = np.ascontiguousarray(Wa[:H])                       # [H, H] f32
    ba_rep = np.ascontiguousarray(
        np.repeat(ba.reshape(HC, P).T, BL, axis=1)          # [128, HC*BL]
    ).astype(np.float32)
    vt = np.ascontiguousarray(v.reshape(H, 1)).astype(_BF16)
    ones_row = np.ones((1, P), np.float32)
    ones_col = np.ones((P, 1), np.float32)
    id128 = np.eye(P, dtype=np.float32)

    in_maps = []
    for c in range(NCORES):
        rows = slice(c * BL, (c + 1) * BL)
        encc = enc[rows]
        in_maps.append({
            "enct": np.ascontiguousarray(encc.transpose(0, 2, 1)).astype(_BF16),
            "encn": encc.astype(_BF16),
            "we": we,
            "wh": wh,
            "ht": np.ascontiguousarray(h[rows].T).astype(np.float32),
            "barep": ba_rep,
            "vt": vt,
            "ones_row": ones_row,
            "ones_col": ones_col,
            "id128": id128,
        })
    return in_maps


def kernel(hidden, encoder_outputs, Wa, ba, v):
    global LAST_RESULT
    from concourse import bass_utils

    nc = get_program()
    in_maps = make_in_maps(hidden, encoder_outputs, Wa, ba, v)
    trace = bool(int(os.environ.get("BASS_TRACE", "0") or "0"))
    res = bass_utils.run_bass_kernel_spmd(
        nc, in_maps, core_ids=list(range(NCORES)), trace=trace
    )
    LAST_RESULT = res
    context = np.concatenate([r["ctx_out"] for r in res.results], axis=0)
    attn = np.concatenate([r["attn_out"] for r in res.results], axis=0)
    return context, attn


# revision 15
# speedup vs baseline: 1.1274x; 1.0826x over previous
"""Trainium2 Bass kernel for nn_Attention (Bahdanau-style attention).

Math (per batch row b):
    energy  = tanh(h[b] @ Wh + enc[b] @ We + ba)        # [S, H]
    scores  = energy @ v                                 # [S]
    attn    = softmax(scores)                            # [S]
    context = attn @ enc[b]                              # [2H]

Sharding: data-parallel over batch B=32 across 8 cores (4 rows/core).
Device layout strategy:
  - The big matmul enc @ We contracts over e (=2H): the PE contracts over
    the partition dim, so enc must be laid out [e, s].  We pre-transpose
    enc on the host and upload encT (bf16) per core; a second natural
    copy (bf16) is uploaded for the final context bmm which contracts
    over s.  All accumulation on-device is fp32 (PSUM), so bf16 only
    affects input rounding (~1e-3 rel err).
  - energyT tiles come out [h, s] so the per-row bias (h@Wh + ba) is a
    per-partition scalar -> fused into the tanh activation on ScalarE.
  - scoresT [s_part, 1] columns are produced directly by PE matmuls
    (lhsT = energyT tile, rhs = v chunk), giving softmax the s-on-
    partitions layout that the context bmm needs for its lhsT.
  - softmax skips max-subtraction: |scores| <= ||v||_1 ~ 11.4, exp is
    safe in fp32.
"""

import os

import numpy as np
import ml_dtypes

B, S, H = 32, 2048, 512
E = 2 * H          # 1024
NCORES = 8
BL = B // NCORES   # 4 batch rows per core
P = 128
ST = 512           # s-tile (one PSUM bank of fp32)
NST = S // ST      # 4
EC = E // P        # 8 e-chunks (contraction of the big matmul)
HC = H // P        # 4 h-chunks
SC = S // P        # 16 s-chunks of 128

_BF16 = ml_dtypes.bfloat16

_PROG = None       # cached Bass program
LAST_RESULT = None # BassKernelResults of the last kernel() call (for test.py)


def _emit(ctx, tc, t):
    """Emit the per-core SPMD program. `t` is the dict of dram APs."""
    import concourse.bass as bass  # noqa: F401
    import concourse.mybir as mybir

    nc = tc.nc
    f32 = mybir.dt.float32
    bf16 = mybir.dt.bfloat16
    Tanh = mybir.ActivationFunctionType.Tanh
    Exp = mybir.ActivationFunctionType.Exp
    X = mybir.AxisListType.X

    consts = ctx.enter_context(tc.tile_pool(name="consts", bufs=1))
    io_encn = ctx.enter_context(tc.tile_pool(name="io_encn", bufs=2))
    io_enct = ctx.enter_context(tc.tile_pool(name="io_enct", bufs=3))
    work = ctx.enter_context(tc.tile_pool(name="work", bufs=2))
    ps_energy = ctx.enter_context(tc.tile_pool(name="ps_energy", bufs=3, space="PSUM"))
    ps_sc = ctx.enter_context(tc.tile_pool(name="ps_sc", bufs=1, space="PSUM"))
    ps_misc = ctx.enter_context(tc.tile_pool(name="ps_misc", bufs=1, space="PSUM"))
    ps_ctx = ctx.enter_context(tc.tile_pool(name="ps_ctx", bufs=1, space="PSUM"))

    # ---- startup loads.  The first mm1 tile is gated on `we` + the first
    # encT slice, so those two go first on the sync queue; everything else
    # that is not needed immediately is either on another queue (small) or
    # gated behind `we` with a tiny DVE corner-write so its descriptors
    # don't steal SDMA bandwidth from the critical loads.
    we_sb = consts.tile([P, EC, H], bf16)
    nc.sync.dma_start(we_sb[:], t["we"].rearrange("(ec p) h -> p ec h", p=P))
    vt_sb = consts.tile([P, HC, 1], bf16)
    nc.scalar.dma_start(vt_sb[:], t["vt"].rearrange("(hc p) o -> p hc o", p=P))
    wh_sb = consts.tile([P, HC, H], f32)
    nc.scalar.dma_start(wh_sb[:], t["wh"].rearrange("(ec p) h -> p ec h", p=P))
    ht_sb = consts.tile([P, HC, BL], f32)
    nc.scalar.dma_start(ht_sb[:], t["ht"].rearrange("(ec p) b -> p ec b", p=P))
    ba_sb = consts.tile([P, HC * BL], f32)
    nc.scalar.dma_start(ba_sb[:], t["barep"])
    ones_row = consts.tile([1, P], f32)
    nc.scalar.dma_start(ones_row[:], t["ones_row"])
    ones_col = consts.tile([P, 1], f32)
    nc.scalar.dma_start(ones_col[:], t["ones_col"])
    id128 = consts.tile([P, P], f32)
    nc.scalar.dma_start(id128[:], t["id128"])
    # PE warm-up: a few dependency-free matmuls run during the first DMA
    # wait and trip the HAM clock gate to 2.4 GHz before real work starts.
    scratch = consts.tile([P, ST], bf16)
    nc.vector.memset(scratch[:], 0.25)
    warm_ps = ps_misc.tile([P, 512], f32, tag="misc", name="warm_ps")
    for _ in range(8):
        nc.tensor.matmul(warm_ps[:], scratch[:, 0:P], scratch[:], start=True, stop=True)

    encn_tiles = [None] * BL
    eng_tiles = {}
    expT_tiles = [None] * BL
    attnb_tiles = [None] * BL
    cps_tiles = [None] * BL
    scT_ps = [None] * BL
    bias_sb = consts.tile([P, HC * BL], f32)

    encT_r = t["enct"].rearrange("b (ec p) s -> b p ec s", p=P)
    encN_r = t["encn"].rearrange("b (sc p) e -> b p sc e", p=P)
    attn_r = t["attn_out"].rearrange("b (q p) -> b q p", p=P)

    def emit_hproj():
        # h_proj[b, h] via a single M=4 matmul chain, then transpose to
        # [h, b] columns for the per-partition tanh bias.
        hp_ps = ps_misc.tile([P, 512], f32, tag="misc", name="hp_ps")
        for ec in range(HC):
            nc.tensor.matmul(
                hp_ps[0:BL, 0:H],
                ht_sb[:, ec, :],
                wh_sb[:, ec, :],
                start=(ec == 0),
                stop=(ec == HC - 1),
            )
        hp_sb = consts.tile([BL, H], f32)
        nc.vector.tensor_copy(hp_sb[:], hp_ps[0:BL, 0:H])
        hpT_ps = ps_misc.tile([P, 512], f32, tag="misc", name="hpT_ps")
        for hc in range(HC):
            nc.tensor.transpose(
                hpT_ps[:, hc * BL:(hc + 1) * BL],
                hp_sb[:, hc * P:(hc + 1) * P],
                id128[0:BL, 0:BL],
            )
        nc.vector.tensor_add(bias_sb[:], hpT_ps[:, 0:HC * BL], ba_sb[:])

    def emit_mm1(b, st):
        et = io_enct.tile([P, EC, ST], bf16, tag="et", name=f"et{b}_{st}")
        if b == 0 and st in (1, 2):
            nc.vector.tensor_copy(et[0:1, 0, 0:1], we_sb[0:1, 0, 0:1])
        nc.sync.dma_start(et[:], encT_r[b, :, :, st * ST:(st + 1) * ST])
        eng = work.tile([P, HC, ST], bf16, tag="eng", bufs=3, name=f"eng{b}_{st}")
        eng_tiles[(b, st)] = eng
        for hc in range(HC):
            pe = ps_energy.tile([P, ST], f32, tag="pe", name=f"pe{b}_{st}_{hc}")
            for ec in range(EC):
                nc.tensor.matmul(
                    pe[:],
                    we_sb[:, ec, hc * P:(hc + 1) * P],
                    et[:, ec, :],
                    start=(ec == 0),
                    stop=(ec == EC - 1),
                )
            if b == 0 and st == 0 and hc == 0:
                emit_hproj()
            nc.scalar.activation(
                eng[:, hc, :], pe[:], Tanh,
                bias=bias_sb[:, hc * BL + b:hc * BL + b + 1],
            )

    def emit_scores(b, st):
        # scoresT columns [128s, 1] (lhsT = energyT chunk, rhs = v chunk),
        # then exp and bf16 cast on the [128, 4] column group.
        for j in range(NST):
            sc = st * NST + j
            for hc in range(HC):
                nc.tensor.matmul(
                    scT_ps[b][:, sc:sc + 1],
                    eng_tiles[(b, st)][:, hc, j * P:(j + 1) * P],
                    vt_sb[:, hc, :],
                    start=(hc == 0),
                    stop=(hc == HC - 1),
                )
        cols = slice(st * NST, (st + 1) * NST)
        nc.scalar.activation(expT_tiles[b][:, cols], scT_ps[b][:, cols], Exp)
        nc.vector.tensor_copy(attnb_tiles[b][:, cols], expT_tiles[b][:, cols])

    def emit_bmm2_chunk(b, st):
        # unnormalized context accumulation; scaled by 1/sum at the end.
        for j in range(NST):
            sc = st * NST + j
            for eh in range(2):
                nc.tensor.matmul(
                    cps_tiles[b][:, eh * ST:(eh + 1) * ST],
                    attnb_tiles[b][:, sc:sc + 1],
                    encn_tiles[b][:, sc, eh * ST:(eh + 1) * ST],
                    start=(sc == 0),
                    stop=(sc == SC - 1),
                )

    def emit_softmax_finish(b):
        expT = expT_tiles[b]
        misc = ps_misc.tile([P, 512], f32, tag="misc", name=f"misc{b}")
        nc.tensor.matmul(misc[0:1, 0:SC], ones_col[:], expT[:], start=True, stop=True)
        ssum = work.tile([1, 1], f32, tag="ssum", name=f"ssum{b}")
        nc.vector.reduce_sum(ssum[:], misc[0:1, 0:SC], axis=X)
        sinv = work.tile([1, 1], f32, tag="sinv", name=f"sinv{b}")
        nc.vector.reciprocal(sinv[:], ssum[:])
        # context: scale the accumulated bmm2 by 1/sum on the way out
        csb = work.tile([1, E], f32, tag="csb", name=f"csb{b}")
        nc.vector.tensor_scalar_mul(csb[:], cps_tiles[b][:], sinv[:])
        nc.sync.dma_start(t["ctx_out"][b:b + 1, :], csb[:])
        # attention-weights output row
        nc.tensor.matmul(misc[:, 16:17], ones_row[:], sinv[:], start=True, stop=True)
        inv_col = work.tile([P, 1], f32, tag="inv_col", name=f"inv{b}")
        nc.vector.tensor_copy(inv_col[:], misc[:, 16:17])
        attnT_f = work.tile([P, SC], f32, tag="attnT_f", name=f"attnTf{b}")
        nc.vector.tensor_scalar_mul(attnT_f[:], expT[:], inv_col[:])
        nc.tensor.transpose(misc[0:SC, 17:17 + P], attnT_f[:], id128[:])
        att_sb = work.tile([SC, P], f32, tag="att_sb", name=f"att{b}")
        nc.vector.tensor_copy(att_sb[:], misc[0:SC, 17:17 + P])
        nc.sync.dma_start(attn_r[b], att_sb[:])

    for b in range(BL):
        encn_t = io_encn.tile([P, SC, E], bf16, tag="encn", name=f"encn{b}")
        if b == 0:
            for q in range(NST):
                nc.vector.tensor_copy(encn_t[0:1, q * NST, 0:1], we_sb[0:1, 0, 0:1])
        elif b == 1:
            for q in range(NST):
                nc.vector.tensor_copy(
                    encn_t[0:1, q * NST, 0:1], eng_tiles[(0, 0)][0:1, 0, 0:1]
                )
        for q in range(NST):
            nc.scalar.dma_start(
                encn_t[:, q * NST:(q + 1) * NST, :],
                encN_r[b, :, q * NST:(q + 1) * NST, :],
            )
        encn_tiles[b] = encn_t
        scT_ps[b] = ps_sc.tile([P, SC], f32, tag="scT", name=f"scT{b}")
        expT_tiles[b] = work.tile([P, SC], f32, tag="expT", name=f"expT{b}")
        attnb_tiles[b] = work.tile([P, SC], bf16, tag="attnb", name=f"attnb{b}")
        cps_tiles[b] = ps_ctx.tile([1, E], f32, tag="cps", name=f"cps{b}")
        for st in range(NST):
            emit_mm1(b, st)
            if st == 1 and b > 0:
                emit_softmax_finish(b - 1)
            if st > 0:
                emit_scores(b, st - 1)
                emit_bmm2_chunk(b, st - 1)
        emit_scores(b, NST - 1)
        emit_bmm2_chunk(b, NST - 1)
    emit_softmax_finish(BL - 1)


def _build_program():
    from contextlib import ExitStack

    import concourse.mybir as mybir
    import concourse.tile as tile
    from concourse import bacc

    f32 = mybir.dt.float32
    bf16 = mybir.dt.bfloat16

    nc = bacc.Bacc("TRN2", debug=False, num_devices=NCORES)
    t = {
        "enct": nc.dram_tensor("enct", [BL, E, S], bf16, kind="ExternalInput").ap(),
        "encn": nc.dram_tensor("encn", [BL, S, E], bf16, kind="ExternalInput").ap(),
        "we": nc.dram_tensor("we", [E, H], bf16, kind="ExternalInput").ap(),
        "wh": nc.dram_tensor("wh", [H, H], f32, kind="ExternalInput").ap(),
        "ht": nc.dram_tensor("ht", [H, BL], f32, kind="ExternalInput").ap(),
        "barep": nc.dram_tensor("barep", [P, HC * BL], f32, kind="ExternalInput").ap(),
        "vt": nc.dram_tensor("vt", [H, 1], bf16, kind="ExternalInput").ap(),
        "ones_row": nc.dram_tensor("ones_row", [1, P], f32, kind="ExternalInput").ap(),
        "ones_col": nc.dram_tensor("ones_col", [P, 1], f32, kind="ExternalInput").ap(),
        "id128": nc.dram_tensor("id128", [P, P], f32, kind="ExternalInput").ap(),
        "ctx_out": nc.dram_tensor("ctx_out", [BL, E], f32, kind="ExternalOutput").ap(),
        "attn_out": nc.dram_tensor("attn_out", [BL, S], f32, kind="ExternalOutput").ap(),
    }
    with tile.TileContext(nc) as tc, ExitStack() as ctx:
        _emit(ctx, tc, t)
    nc.compile()
    return nc


def get_program():
    global _PROG
    if _PROG is None:
        _PROG = _build_program()
    return _PROG


def make_in_maps(hidden, encoder_outputs, Wa, ba, v):
    hidden = np.asarray(hidden, dtype=np.float32)
    enc = np.asarray(encoder_outputs, dtype=np.float32)
    Wa = np.asarray(Wa, dtype=np.float32)
    ba = np.asarray(ba, dtype=np.float32)
    v = np.asarray(v, dtype=np.float32)

    h = hidden[0]                       # [B, H]
    we = np.ascontiguousarray(Wa[H:]).astype(_BF16)         # [E, H]
    wh = np.ascontiguousarray(Wa[:H])                       # [H, H] f32
    ba_rep = np.ascontiguousarray(
        np.repeat(ba.reshape(HC, P).T, BL, axis=1)          # [128, HC*BL]
    ).astype(np.float32)
    vt = np.ascontiguousarray(v.reshape(H, 1)).astype(_BF16)
    ones_row = np.ones((1, P), np.float32)
    ones_col = np.ones((P, 1), np.float32)
    id128 = np.eye(P, dtype=np.float32)

    in_maps = []
    for c in range(NCORES):
        rows = slice(c * BL, (c + 1) * BL)
        encc = enc[rows]
        in_maps.append({
            "enct": np.ascontiguousarray(encc.transpose(0, 2, 1)).astype(_BF16),
            "encn": encc.astype(_BF16),
            "we": we,
            "wh": wh,
            "ht": np.ascontiguousarray(h[rows].T).astype(np.float32),
            "barep": ba_rep,
            "vt": vt,
            "ones_row": ones_row,
            "ones_col": ones_col,
            "id128": id128,
        })
    return in_maps


def kernel(hidden, encoder_outputs, Wa, ba, v):
    global LAST_RESULT
    from concourse import bass_utils

    nc = get_program()
    in_maps = make_in_maps(hidden, encoder_outputs, Wa, ba, v)
    trace = bool(int(os.environ.get("BASS_TRACE", "0") or "0"))
    res = bass_utils.run_bass_kernel_spmd(
        nc, in_maps, core_ids=list(range(NCORES)), trace=trace
    )
    LAST_RESULT = res
    context = np.concatenate([r["ctx_out"] for r in res.results], axis=0)
    attn = np.concatenate([r["attn_out"] for r in res.results], axis=0)
    return context, attn
